# revision 12
# baseline (speedup 1.0000x reference)
"""Trainium2 Bass kernel for the CNN_PHMM_VAE loss (pHMM forward algorithm + KLD).

Strategy
--------
Pure data parallel over batch: each of the 8 cores processes 512 batch rows.
Per core, batch rows live on 128 SBUF partitions x 4 groups packed along the
free axis (group stride 66 = 65 motif states + 1 pad column).

The forward recurrence runs in *scaled exponential space*.  State variables are
pre-multiplied by transition factors so the 3-term "prev" combination becomes
pure adds:

    GM[k] = exp(a_M2M[k]) * FM[k]
    GI[k] = exp(a_I2M[k]) * FI[k]
    GD[k] = exp(a_D2M[k]) * FD[k]
    prev[k] = GM[k] + GI[k] + GD[k]           (= exp-space M/I/D -> M input)

Per l-step the host streams one [128, 2F] bf16 tile holding two tables:
TABS[c] (match update, pre-shifted) and TAB2[c] = R[c]*TABS[c-1] (delete-chain
source, shift-by-2 so its operands stay even-aligned):

    GpSimd:  t1   = C1 . GM         full-tile ops only (GpSimd runs sliced
             t2   = C2 . GI         access patterns at ~2x cost)
             GI'  = t1 + t2
    Vector:  s    = GM + GI
             PV[2:F+2] = s + GD                       (prev, left-padded)
             GM'[c] = TABS[c] . PV[c+1]               (the one odd-aligned op)
             SD1[c] = TAB2[c] . PV[c]                 (even aligned)
             GD'  = tensor_tensor_scan(ATIL, SD1)     (delete-state chain)

DVE ops with odd element offsets lose the 16-bit packed mode (measured ~594 ns
odd-input and ~1272 ns odd-output vs ~294 ns aligned for [128, 264] bf16), so
the layout keeps every op except GM' on even offsets: prev lives in a [128,
F+2] buffer whose two dead leading columns make SD1's shift-by-2 read even,
and the group pad columns of TABS/TAB2 are zero so flat contiguous ops
implement the per-group k-shift with no 3D strided views (~776 ns).  Zeros in
ATIL/TAB2 at group boundaries reset the scan so four batch groups share one
scan instruction.

Every 32 steps the state is rescaled by the power-of-two floor of prev's
per-row max (a valid scale within e^6 of the true state max; log accumulated
in f32 CACC).  The per-step tables are pre-scaled by exp(-c_b) (c_b ~ the
row's mean log growth) so 32-step windows stay inside bf16 range; every path
consumes exactly one of {TABS, C1, C2} per step, so this is an exact uniform
rescaling corrected once at readout (+L*c_b per row, applied host-side).

The emission gather e[b, k, x[b,l]] is a pure data reindex done host-side when
building the TABS/TAB2 stream.

Final: -log P = -(ln(prev[64]) + c_acc + L*c_b); ln computed as ln(mantissa) +
e*ln2 (the Act engine's Ln table is only accurate near O(1) inputs); KLD
reduced on-device; one [1,2] f32 partial per core, combined on host.
"""

import os
import sys

import numpy as np

if "/opt/trn_rl_repo" not in sys.path:
    sys.path.insert(0, "/opt/trn_rl_repo")

import ml_dtypes

BF16 = np.dtype(ml_dtypes.bfloat16)

# problem constants (hardcoded per the harness contract)
B, K, L, E = 4096, 64, 128, 16
NCORES = 8
BPC = B // NCORES          # 512 batch rows per core
G = BPC // 128             # 4 groups
GS = K + 2                 # group stride 66 (65 states + 1 pad)
F = G * GS                 # 264 free columns of state
RESCALE_EVERY = 32

_CACHE = {}


def _build_program():
    import concourse.bacc as bacc
    import concourse.mybir as mybir
    from concourse.tile import TileContext

    bf16 = mybir.dt.bfloat16
    f32 = mybir.dt.float32
    i32 = mybir.dt.int32
    MULT = mybir.AluOpType.mult
    ADD = mybir.AluOpType.add
    MAX = mybir.AluOpType.max
    SUB = mybir.AluOpType.subtract
    AND = mybir.AluOpType.bitwise_and
    OR = mybir.AluOpType.bitwise_or
    LSR = mybir.AluOpType.logical_shift_right
    X = mybir.AxisListType.X
    ACT = mybir.ActivationFunctionType

    nc = bacc.Bacc("TRN2", target_bir_lowering=False, debug=False,
                   num_devices=NCORES)

    tab_h = nc.declare_dram_parameter("tab", [L, 128, 2 * F], bf16,
                                      isOutput=False)
    c1_h = nc.declare_dram_parameter("c1", [128, F], bf16, isOutput=False)
    c2_h = nc.declare_dram_parameter("c2", [128, F], bf16, isOutput=False)
    atil_h = nc.declare_dram_parameter("atil", [128, F], bf16, isOutput=False)
    gm0_h = nc.declare_dram_parameter("gm0", [128, F], bf16, isOutput=False)
    gd0_h = nc.declare_dram_parameter("gd0", [128, F], bf16, isOutput=False)
    cinit_h = nc.declare_dram_parameter("cinit", [128, G], f32, isOutput=False)
    mus_h = nc.declare_dram_parameter("mus", [128, G * E], f32, isOutput=False)
    lv_h = nc.declare_dram_parameter("lv", [128, G * E], f32, isOutput=False)
    out_h = nc.declare_dram_parameter("out", [1, 2], f32, isOutput=True)

    with TileContext(nc) as tc:
        with tc.tile_pool(name="consts", bufs=1) as consts, \
             tc.tile_pool(name="state", bufs=1) as state, \
             tc.tile_pool(name="tmps", bufs=2) as tmps, \
             tc.tile_pool(name="stream", bufs=12) as stream, \
             tc.tile_pool(name="psum", bufs=1, space="PSUM") as psum_pool:

            C1 = consts.tile([128, F], bf16)
            nc.sync.dma_start(C1[:], c1_h[:])
            C2 = consts.tile([128, F], bf16)
            nc.sync.dma_start(C2[:], c2_h[:])
            ATIL = consts.tile([128, F], bf16)
            nc.sync.dma_start(ATIL[:], atil_h[:])
            MUS = consts.tile([128, G * E], f32)
            nc.sync.dma_start(MUS[:], mus_h[:])
            LV = consts.tile([128, G * E], f32)
            nc.sync.dma_start(LV[:], lv_h[:])

            GM = [state.tile([128, F], bf16, name=f"gm{i}") for i in (0, 1)]
            GI = [state.tile([128, F], bf16, name=f"gi{i}") for i in (0, 1)]
            GD = state.tile([128, F], bf16)
            SD = state.tile([128, F], bf16)
            PVs = [state.tile([128, F + 2], bf16, name=f"pv{i}")
                   for i in (0, 1)]
            nc.sync.dma_start(GM[0][:], gm0_h[:])
            nc.sync.dma_start(GD[:], gd0_h[:])
            nc.vector.memset(GI[0][:], 0.0)
            nc.vector.memset(PVs[0][:], 0.0)
            nc.vector.memset(PVs[1][:], 0.0)
            CACC = state.tile([128, G], f32)
            nc.sync.dma_start(CACC[:], cinit_h[:])

            def g3(t):
                return t.rearrange("p (g k) -> p g k", g=G)

            H = F // 2  # column split at a group boundary (groups 0-1 | 2-3)
            for l in range(L):
                r, w = l % 2, 1 - (l % 2)
                PV = PVs[w]

                tab = stream.tile([128, 2 * F], bf16, name="tab", tag="tab")
                nc.sync.dma_start(tab[:], tab_h[l])

                # GpSimd: t1 and the final GI' add (t2 runs on Vector)
                t1 = tmps.tile([128, F], bf16, name="t1", tag="t1")
                nc.gpsimd.tensor_tensor(t1[:], C1[:], GM[r][:], MULT)

                # Vector: full-width ops with t2-halves as spacers so no DVE
                # op depends on its immediate predecessor (a back-to-back
                # RAW/WAW costs a ~440 ns pipeline drain; independent
                # neighbors overlap and issue at ~206 ns per [128, 264]).
                s = tmps.tile([128, F], bf16, name="s", tag="s")
                t2 = tmps.tile([128, F], bf16, name="t2", tag="t2")
                nc.vector.tensor_tensor(s[:], GM[r][:], GI[r][:], ADD)
                nc.vector.tensor_tensor(
                    t2[:, 0:H], C2[:, 0:H], GI[r][:, 0:H], MULT)
                nc.vector.tensor_tensor(PV[:, 2:F + 2], s[:], GD[:], ADD)
                nc.vector.tensor_tensor(
                    t2[:, H:F], C2[:, H:F], GI[r][:, H:F], MULT)
                nc.vector.tensor_tensor(
                    SD[:], tab[:, F:2 * F], PV[:, 0:F], MULT)
                nc.vector.tensor_tensor(
                    GM[w][:, 0:H], tab[:, 0:H], PV[:, 1:H + 1], MULT)
                nc.vector.tensor_tensor(
                    GM[w][:, H:F], tab[:, H:F], PV[:, H + 1:F + 1], MULT)
                nc.vector.tensor_tensor_scan(
                    GD[:], ATIL[:], SD[:], 0.0, MULT, ADD)

                # GI' = t1 + t2 on GpSimd (consumed next step)
                nc.gpsimd.tensor_tensor(GI[w][:], t1[:], t2[:], ADD)

                if l % RESCALE_EVERY == RESCALE_EVERY - 1 and l != L - 1:
                    # scale by pow2(max of prev) -- any consistently-logged
                    # scale is exact; prev's max is within e^6 of the states'
                    pv3 = PV[:, 2:F + 2].rearrange("p (g k) -> p g k", g=G)
                    rm = tmps.tile([128, G], f32, name="rm", tag="rm")
                    nc.vector.tensor_reduce(rm[:], pv3, X, MAX)
                    nc.vector.tensor_scalar_max(rm[:], rm[:], 1e-30)
                    mask = tmps.tile([128, G], i32, name="mask", tag="mask")
                    nc.vector.tensor_scalar(
                        mask[:], rm.bitcast(i32), 0x7F800000, None, AND)
                    rib = tmps.tile([128, G], i32, name="rib", tag="rib")
                    nc.vector.tensor_scalar(
                        rib[:], mask[:], -1, 0x7F000000, MULT, ADD)
                    rinv = tmps.tile([128, G], f32, name="rinv", tag="rinv")
                    nc.vector.tensor_copy(rinv.bitcast(i32), rib[:])
                    es = tmps.tile([128, G], i32, name="es", tag="es")
                    nc.vector.tensor_scalar(es[:], mask[:], 23, None, LSR)
                    ef = tmps.tile([128, G], f32, name="ef", tag="ef")
                    nc.vector.tensor_copy(ef[:], es[:])
                    el = tmps.tile([128, G], f32, name="el", tag="el")
                    nc.vector.tensor_scalar(
                        el[:], ef[:], 127.0, float(np.log(2.0)), SUB, MULT)
                    nc.vector.tensor_tensor(CACC[:], CACC[:], el[:], ADD)
                    GM3, GI3, GD3 = g3(GM[w]), g3(GI[w]), g3(GD)
                    for g in range(G):
                        sc = rinv[:, g:g + 1]
                        for T3 in (GM3, GI3, GD3):
                            nc.vector.tensor_scalar_mul(
                                T3[:, g, 0:K + 1], T3[:, g, 0:K + 1], sc)

            # ---- final readout ----
            fin = L % 2  # buffer written by the last step
            PV = PVs[1]
            s = tmps.tile([128, F], bf16, name="s", tag="s")
            nc.vector.tensor_tensor(s[:], GM[fin][:], GI[fin][:], ADD)
            nc.vector.tensor_tensor(PV[:, 2:F + 2], s[:], GD[:], ADD)
            pf = tmps.tile([128, G], f32, name="pf", tag="pf")
            # floor before log so a fully-underflowed row cannot produce -inf
            pv3f = PV[:, 2:F + 2].rearrange("p (g k) -> p g k", g=G)
            nc.vector.tensor_scalar_max(pf[:], pv3f[:, :, K], 1e-38)
            # ln(pf) = ln(mantissa in [1,2)) + (exp-127)*ln2: the Act Ln table
            # is only accurate near O(1) inputs, and pf spans e^+-55 here.
            mant = tmps.tile([128, G], f32, name="mant", tag="mant")
            nc.vector.tensor_scalar(
                mant.bitcast(i32), pf.bitcast(i32),
                0x007FFFFF, 0x3F800000, AND, OR)
            pe = tmps.tile([128, G], i32, name="pe", tag="pe")
            nc.vector.tensor_scalar(pe[:], pf.bitcast(i32), 23, None, LSR)
            pef = tmps.tile([128, G], f32, name="pef", tag="pef")
            nc.vector.tensor_copy(pef[:], pe[:])
            pel = tmps.tile([128, G], f32, name="pel", tag="pel")
            nc.vector.tensor_scalar(
                pel[:], pef[:], 127.0, float(np.log(2.0)), SUB, MULT)
            lnp = tmps.tile([128, G], f32, name="lnp", tag="lnp")
            nc.scalar.activation(lnp[:], mant[:], ACT.Ln)

            BOTH = consts.tile([128, 2 * G], f32)
            nc.vector.tensor_tensor(BOTH[:, 0:G], lnp[:], pel[:], ADD)
            nc.vector.tensor_tensor(BOTH[:, 0:G], BOTH[:, 0:G], CACC[:], ADD)

            # KLD pieces: sum_e (logvar - mu^2 - exp(logvar))
            sq = consts.tile([128, G * E], f32)
            nc.scalar.activation(sq[:], MUS[:], ACT.Square)
            elv = consts.tile([128, G * E], f32)
            nc.scalar.activation(elv[:], LV[:], ACT.Exp)
            d1 = consts.tile([128, G * E], f32)
            nc.vector.tensor_sub(d1[:], LV[:], sq[:])
            nc.vector.tensor_sub(d1[:], d1[:], elv[:])
            d13 = d1.rearrange("p (g e) -> p g e", g=G)
            nc.vector.tensor_reduce(BOTH[:, G:2 * G], d13, X, ADD)

            B3 = BOTH.rearrange("p (h g) -> p h g", h=2)
            both2 = consts.tile([128, 2], f32)
            nc.vector.tensor_reduce(both2[:], B3, X, ADD)

            ones = consts.tile([128, 1], f32)
            nc.vector.memset(ones[:], 1.0)
            acc = psum_pool.tile([1, 2], f32)
            nc.tensor.matmul(acc[:], ones[:], both2[:])
            res = consts.tile([1, 2], f32)
            nc.vector.tensor_copy(res[:], acc[:])
            nc.sync.dma_start(out_h[:], res[:])

    nc.compile()
    return nc


def _to_pg(arr):
    """[B, ...] -> [NCORES, 128, G, ...]  with b = c*BPC + g*128 + p."""
    tail = arr.shape[1:]
    return arr.reshape(NCORES, G, 128, *tail).transpose(
        0, 2, 1, *range(3, 3 + len(tail)))


def _pad_state(a65):
    """[B, 65] -> [B, 66] with zero pad column."""
    out = np.zeros((a65.shape[0], GS), a65.dtype)
    out[:, :K + 1] = a65
    return out


def _host_prep(batch_input, transition_probs, emission_probs, mus, logvars):
    x = np.asarray(batch_input, np.int32)
    a = np.asarray(transition_probs, np.float32)
    e = np.asarray(emission_probs, np.float32)
    mus = np.asarray(mus, np.float32)
    lv = np.asarray(logvars, np.float32)

    aM2M, aM2I, aM2D = a[:, :, 0], a[:, :, 1], a[:, :, 2]
    aI2M, aI2I = a[:, :, 3], a[:, :, 4]
    aD2M, aD2D = a[:, :, 5], a[:, :, 6]

    C1 = 0.25 * np.exp(aI2M + aM2I - aM2M)                     # [B,65]
    C2 = 0.25 * np.exp(aI2I)                                   # [B,65]
    ATIL = np.zeros((B, GS), np.float32)
    ATIL[:, 1:K + 1] = np.exp(
        aD2D[:, 0:K] + aD2M[:, 1:K + 1] - aD2M[:, 0:K])
    # delete-chain source coefficient: GD'[j] = ATIL[j]*GD'[j-1] + R[j]*GM'[j-1]
    # R[j] = exp(aD2M[j] + aM2D[j-1] - aM2M[j-1]) for j=2..K, 0 elsewhere
    R2 = np.zeros((B, GS), np.float32)
    R2[:, 2:K + 1] = np.exp(
        aD2M[:, 2:K + 1] + aM2D[:, 1:K] - aM2M[:, 1:K])

    # emission tables premultiplied by the next match transition, then
    # gathered by the observed symbols (pure reindex over input data)
    ehat = np.exp(aM2M[:, 1:K + 1, None] + e)                  # [B,K,4]
    TE = ehat[np.arange(B)[:, None, None],
              np.arange(K)[None, :, None],
              x[:, None, :]]                                   # [B,K,L]
    # drift centering: every path takes exactly one of {TABS, C1, C2} per
    # l-step, so scaling all three by exp(-c_b) scales the state uniformly
    # by exp(-c_b*l).  c_b ~ the row's mean log growth keeps the 32-step
    # windows inside bf16 range (peak log-max ~73 vs overflow at 88.7).
    cb = np.log(TE).mean(axis=(1, 2)) + 1.0                    # [B]
    TE = TE * np.exp(-cb)[:, None, None]
    C1 = C1 * np.exp(-cb)[:, None]
    C2 = C2 * np.exp(-cb)[:, None]

    # pre-shifted tables (group pads zero):
    #   TABS[j] = TE[j-1]        (GM'[j] = TABS[j]*prev[j-1])
    #   TAB2[j] = R[j]*TE[j-2]   (SD1[j] = TAB2[j]*prev[j-2])
    TABS = np.zeros((B, GS, L), np.float32)
    TABS[:, 1:K + 1, :] = TE
    TAB2 = np.zeros((B, GS, L), np.float32)
    TAB2[:, 2:K + 1, :] = R2[:, 2:K + 1, None] * TE[:, 0:K - 1, :]

    # initial state in log space (single-path delete chain), normalized
    gm0_log = np.full((B, K + 1), -np.inf, np.float32)
    gm0_log[:, 0] = aM2M[:, 0]
    fd0 = np.full((B, K + 1), -np.inf, np.float64)
    fd0[:, 1] = aM2D[:, 0]
    fd0[:, 2:] = aM2D[:, 0:1] + np.cumsum(
        aD2D[:, 1:K].astype(np.float64), axis=1)
    gd0_log = fd0 + aD2M
    gd0_log[:, 0] = -np.inf

    cinit = np.maximum(gm0_log.max(axis=1),
                       gd0_log.max(axis=1).astype(np.float32))  # [B]
    GM0 = np.exp(gm0_log - cinit[:, None]).astype(np.float32)
    GD0 = np.exp(gd0_log - cinit[:, None]).astype(np.float32)

    in_maps = []
    c1_pg = _to_pg(_pad_state(C1)).reshape(NCORES, 128, F).astype(BF16)
    c2_pg = _to_pg(_pad_state(C2)).reshape(NCORES, 128, F).astype(BF16)
    atil_pg = _to_pg(ATIL).reshape(NCORES, 128, F).astype(BF16)
    gm0_pg = _to_pg(_pad_state(GM0)).reshape(NCORES, 128, F).astype(BF16)
    gd0_pg = _to_pg(_pad_state(GD0)).reshape(NCORES, 128, F).astype(BF16)
    cinit_pg = _to_pg(cinit[:, None]).reshape(NCORES, 128, G).astype(np.float32)
    mus_pg = _to_pg(mus).reshape(NCORES, 128, G * E).astype(np.float32)
    lv_pg = _to_pg(lv).reshape(NCORES, 128, G * E).astype(np.float32)
    # [B,GS,L] -> [c, L, 128, G*GS], then concat TABS||TAB2 on the free axis
    t1_pg = TABS.reshape(NCORES, G, 128, GS, L).transpose(0, 4, 2, 1, 3) \
        .reshape(NCORES, L, 128, F)
    t2_pg = TAB2.reshape(NCORES, G, 128, GS, L).transpose(0, 4, 2, 1, 3) \
        .reshape(NCORES, L, 128, F)
    tab_pg = np.concatenate([t1_pg, t2_pg], axis=3).astype(BF16)

    for c in range(NCORES):
        in_maps.append({
            "tab": np.ascontiguousarray(tab_pg[c]),
            "c1": np.ascontiguousarray(c1_pg[c]),
            "c2": np.ascontiguousarray(c2_pg[c]),
            "atil": np.ascontiguousarray(atil_pg[c]),
            "gm0": np.ascontiguousarray(gm0_pg[c]),
            "gd0": np.ascontiguousarray(gd0_pg[c]),
            "cinit": np.ascontiguousarray(cinit_pg[c]),
            "mus": np.ascontiguousarray(mus_pg[c]),
            "lv": np.ascontiguousarray(lv_pg[c]),
        })
    return in_maps, float(cb.sum())


def kernel(batch_input, transition_probs, emission_probs, mus, logvars,
           _trace=False, _trace_kwargs=None):
    from concourse.bass_utils import run_bass_kernel_spmd

    if "nc" not in _CACHE:
        _CACHE["nc"] = _build_program()
    nc = _CACHE["nc"]

    in_maps, cb_sum = _host_prep(batch_input, transition_probs,
                                 emission_probs, mus, logvars)
    kw = {}
    if _trace:
        kw["trace"] = True
        kw.update(_trace_kwargs or {})
    res = run_bass_kernel_spmd(nc, in_maps, list(range(NCORES)), **kw)
    _CACHE["last_results"] = res

    total = 0.0
    for c in range(NCORES):
        s0, s1 = np.asarray(res.results[c]["out"], np.float64).ravel()
        total += -s0 - 0.5 * s1 - 8.0 * BPC
    total -= L * cb_sum
    return np.float32(total / B)
